# revision 15
# baseline (speedup 1.0000x reference)
"""Trainium2 Bass kernel for a pre-LN transformer encoder layer.

Contract: kernel(**inputs) takes the FULL inputs (x [1,4096,1024] plus
weights/biases) and returns the FULL output [1,4096,1024].

Sharding: sequence-parallel over 8 NeuronCores. Each core owns 512 rows of
the sequence: it computes LN1, its Q/K/V rows, AllGathers K^T and V in 8
head-pair chunks (pipelined under attention), runs full 16-head attention
for its 512 queries, output projection + residual, LN2, and the FFN.

Perf notes vs the previous version:
- All matmuls feed the PE with 128-deep contraction so the HAM clock
  monitor holds the 2.4GHz state (scores use zero/tiny-padded Q halves,
  ctx uses 128-col padded V tiles with a pre-set ones column).
- Weights and attention activations are bf16 (half the HBM/D2D bytes).
- The KV AllGather is split into 8 per-head-pair chunks issued as soon as
  each chunk of K/V is projected, so transfers overlap attention.
- LayerNorm uses bn_stats/bn_aggr + an ACT-engine output pass.
- A fraction of softmax exps run on the Vector engine via an
  exponent-bit-construction trick to unload the Scalar engine.
"""

import numpy as np
from contextlib import ExitStack

import concourse.bass as bass
import concourse.mybir as mybir
import concourse.tile as tile
from concourse import bacc
from concourse.bass_utils import run_bass_kernel_spmd
from concourse.masks import make_identity

P = 128
NCORES = 8
S = 4096
SL = S // NCORES          # 512 local rows
D = 1024
H = 16
DK = D // H               # 64
F = 4096
EPS = 1e-6

F32 = mybir.dt.float32
BF16 = mybir.dt.bfloat16
I16 = mybir.dt.int16
AF = mybir.ActivationFunctionType
OP = mybir.AluOpType

KN = P * SL               # K chunk elems per (hh, core): [128 dk, 512 keys]
VN = SL * P               # V chunk elems per (hh, core): [512 keys, 128 dims]
CH = KN + VN              # 131072 elems (bf16) per chunk per core

TINY = 1e-30              # mirror scale for the padded Q halves
LOG2E_128 = 184.6649652337873   # 128 * log2(e)
EXPBIAS = 16256.0               # 127 * 128
SPILL_MOD = 4             # every SPILL_MOD-th score tile exps on DVE

_CACHE = {}


def _build(ln1_a, ln1_b, ln2_a, ln2_b):
    nc = bacc.Bacc("TRN2", target_bir_lowering=False, debug=False,
                   num_devices=NCORES)

    x_d = nc.dram_tensor("x_loc", [SL, D], F32, kind="ExternalInput")
    wq_d = nc.dram_tensor("Wq", [D, D], BF16, kind="ExternalInput")
    wk_d = nc.dram_tensor("Wk", [D, D], BF16, kind="ExternalInput")
    wv_d = nc.dram_tensor("Wv", [D, D], BF16, kind="ExternalInput")
    wo_d = nc.dram_tensor("Wo", [D, D], BF16, kind="ExternalInput")
    w1_d = nc.dram_tensor("W1", [D, F], BF16, kind="ExternalInput")
    w2_d = nc.dram_tensor("W2", [F, D], BF16, kind="ExternalInput")
    bq_d = nc.dram_tensor("bq", [D], F32, kind="ExternalInput")
    bk_d = nc.dram_tensor("bk", [D], F32, kind="ExternalInput")
    b1_d = nc.dram_tensor("b1", [F], F32, kind="ExternalInput")
    bx3_d = nc.dram_tensor("bx3", [3, D], BF16, kind="ExternalInput")
    y_d = nc.dram_tensor("y_loc", [SL, D], F32, kind="ExternalOutput")

    with tile.TileContext(nc) as tc, ExitStack() as ctx:
        const = ctx.enter_context(tc.tile_pool(name="const", bufs=1))
        stat = ctx.enter_context(tc.tile_pool(name="stat", bufs=16))
        tmp = ctx.enter_context(tc.tile_pool(name="tmp", bufs=2))
        dram = ctx.enter_context(tc.tile_pool(name="dram", bufs=1, space="DRAM"))

        # ---------------- constants ----------------
        ident = const.tile([P, P], BF16)
        make_identity(nc, ident)
        ones65 = const.tile([65, P], BF16)
        nc.vector.memset(ones65[:], 1.0)
        ones1 = ones65[0:1, :]
        heat_a = const.tile([P, P], BF16)
        nc.vector.memset(heat_a[:], 0.5)
        hb_pool = ctx.enter_context(tc.tile_pool(name="hb_pool", bufs=1))

        def heat_burst(ps_pool, n, rhs, nm):
            """n back-to-back 128x128x512 matmuls: a dense >=3.4us burst
            flips the PE HAM to 2.4GHz; `rhs` gates when the burst runs."""
            hp = ps_pool.tile([P, SL], F32, name=f"heat_{nm}", tag="heat")
            for i in range(n):
                nc.tensor.matmul(hp[:], heat_a[:], rhs, start=True, stop=True)

        # E65[k, m]: row 0 selects m<64 (head A), row 64 selects m>=64 (B)
        e65 = const.tile([65, P], BF16)
        nc.vector.memset(e65[:], 0.0)
        nc.vector.memset(e65[0:1, 0:64], 1.0)
        nc.vector.memset(e65[64:65, 64:128], 1.0)
        rc65_f = const.tile([65, SL], F32)
        nc.vector.memset(rc65_f[:], 1.0)

        bq_t = const.tile([P, 8], F32)
        nc.sync.dma_start(bq_t[:], bq_d.rearrange("(c p) -> p c", p=P))
        bq8 = const.tile([P, 8], F32)
        nc.vector.tensor_scalar(bq8[:], bq_t[:], 0.125, None, OP.mult)
        bk_t = const.tile([P, 8], F32)
        nc.sync.dma_start(bk_t[:], bk_d.rearrange("(c p) -> p c", p=P))
        b1_t = const.tile([P, 32], F32)
        nc.sync.dma_start(b1_t[:], b1_d.rearrange("(c p) -> p c", p=P))

        rcon = const.tile([65, D], BF16)
        nc.sync.dma_start(rcon[0:1, :], bx3_d[0:1, :])
        nc.sync.dma_start(rcon[32:33, :], bx3_d[1:2, :])
        nc.sync.dma_start(rcon[64:65, :], bx3_d[2:3, :])
        bvr = rcon[0:1, :]
        bor = rcon[32:33, :]
        b2r = rcon[64:65, :]

        def layer_norm_to_T(src_big, a_val, b_val, hT, tp_psum,
                            burst_pool=None):
            """src_big [P, 4, D] F32 -> hT [P, 8, SL] BF16 (transposed LN).

            Stats via bn_stats/bn_aggr (one DVE pass), normalization applied
            on the ACT engine (Copy with per-partition scale/bias), then PE
            transposes in bf16.
            """
            for j in range(4):
                st = stat.tile([P, 2, 6], F32, name=f"bst{j}", tag="bst")
                for hhalf in range(2):
                    nc.vector.bn_stats(
                        st[:, hhalf, :],
                        src_big[:, j, hhalf * 512:(hhalf + 1) * 512])
                mv = stat.tile([P, 2], F32, name=f"mv{j}", tag="mv")
                nc.vector.bn_aggr(mv[:], st[:])
                rr = stat.tile([P, 1], F32, name=f"rr{j}", tag="rr")
                nc.scalar.activation(rr[:], mv[:, 1:2], AF.Sqrt,
                                     scale=float(D) / (D - 1))
                nc.vector.tensor_scalar_add(rr[:], rr[:], EPS)
                nc.vector.reciprocal(rr[:], rr[:])
                nc.vector.tensor_scalar_mul(rr[:], rr[:], float(a_val))
                bp = stat.tile([P, 1], F32, name=f"bp{j}", tag="bp")
                nc.vector.tensor_tensor(bp[:], mv[:, 0:1], rr[:], OP.mult)
                nc.vector.tensor_scalar(bp[:], bp[:], -1.0, float(b_val),
                                        OP.mult, OP.add)
                h = tmp.tile([P, D], BF16, name=f"h{j}", tag="h")
                nc.scalar.activation(h[:], src_big[:, j, :], AF.Identity,
                                     bias=bp[:], scale=rr[:])
                if burst_pool is not None and j in (0, 2):
                    heat_burst(burst_pool, 6, h[:, 0:SL], f"ln{j}")
                for half in range(2):
                    tp = tp_psum.tile([P, 512], BF16, name=f"tp{j}_{half}",
                                      tag="tp")
                    for k in range(4):
                        cc = half * 4 + k
                        nc.tensor.transpose(tp[:, k * P:(k + 1) * P],
                                            h[:, cc * P:(cc + 1) * P],
                                            ident[:])
                    nc.vector.tensor_copy(
                        hT[:, half * 4:(half + 1) * 4, j * P:(j + 1) * P],
                        tp.rearrange("p (c q) -> p c q", q=P))

        groups = [list(range(NCORES))]
        CHUNKS = [(0,), (1,), (2, 3), (4, 5), (6, 7)]
        CHUNK_OF = {hh: gi for gi, hhs in enumerate(CHUNKS) for hh in hhs}
        CHUNK_OFF = {hh: hhs.index(hh) for hhs in CHUNKS for hh in hhs}
        KVCs = [dram.tile([len(hhs) * CH], BF16, name=f"kvc{gi}")
                for gi, hhs in enumerate(CHUNKS)]
        GKVs = [dram.tile([NCORES * len(hhs) * CH], BF16, name=f"gkv{gi}",
                          addr_space="Shared") for gi, hhs in enumerate(CHUNKS)]

        # W1 stream pool: first used right after phase 2, so the DMAs
        # prefetch during attention.
        w1pool = ctx.enter_context(tc.tile_pool(name="w1pool", bufs=24))
        x2_pool = ctx.enter_context(tc.tile_pool(name="x2_pool", bufs=1))

        with (
            tc.tile_pool(name="x_pool", bufs=1) as x_pool,
            tc.tile_pool(name="ctx_pool", bufs=1) as ctx_pool,
        ):
            x_big = x_pool.tile([P, 4, D], F32)
            with tc.tile_pool(name="qt_pool", bufs=1) as qt_pool:
                QTA = qt_pool.tile([P, 8, SL], BF16, name="QTA")
                QTB = qt_pool.tile([P, 8, SL], BF16, name="QTB")

                # ---------------- phase 1: LN1 + transpose ----------------
                with tc.tile_pool(name="hT_pool", bufs=1) as hT_pool:
                    hT = hT_pool.tile([P, 8, SL], BF16)
                    with tc.tile_pool(name="tp1", bufs=2, space="PSUM") as tpp:
                        for j in range(4):
                            nc.sync.dma_start(x_big[:, j, :],
                                              x_d[j * P:(j + 1) * P, :])
                        layer_norm_to_T(x_big, ln1_a, ln1_b, hT, tpp,
                                        burst_pool=tpp)
                        heat_burst(tpp, 6, hT[:, 4, :], "p2")

                    # ------- phase 2: hh-major K/V/Q + chunked gathers -----
                    with (
                        tc.tile_pool(name="wbig", bufs=25) as wbig,
                        tc.tile_pool(name="kvstage", bufs=2) as kvstage,
                        tc.tile_pool(name="qkps", bufs=2, space="PSUM") as qkps,
                    ):
                        wkt, wvt, wqt = [], [], []
                        for nm, wd, lst in (("wk", wk_d, wkt),
                                            ("wv", wv_d, wvt),
                                            ("wq", wq_d, wqt)):
                            for cc in range(8):
                                w = wbig.tile([P, D], BF16, name=f"{nm}{cc}",
                                              tag="wbig")
                                nc.sync.dma_start(
                                    w[:], wd[cc * P:(cc + 1) * P, :])
                                lst.append(w)

                        for hh in range(8):
                            hs = slice(hh * P, (hh + 1) * P)
                            # K chunk: [128 dk, 512 keys]
                            ps = qkps.tile([P, SL], F32, name=f"kps{hh}",
                                           tag="qk")
                            for cc in range(8):
                                nc.tensor.matmul(
                                    ps[:], wkt[cc][:, hs], hT[:, cc, :],
                                    start=(cc == 0), stop=(cc == 7))
                            kstg = kvstage.tile([P, SL], BF16,
                                                name=f"kstg{hh}", tag="kstg")
                            nc.scalar.activation(kstg[:], ps[:], AF.Identity,
                                                 bias=bk_t[:, hh:hh + 1])
                            cb = KVCs[CHUNK_OF[hh]][CHUNK_OFF[hh] * CH:]
                            nc.sync.dma_start(
                                cb[0:KN].rearrange("(d q) -> d q", q=SL),
                                kstg[:])
                            # V chunk: [512 keys, 128 dims]
                            vstg = kvstage.tile([P, 4, P], BF16,
                                                name=f"vstg{hh}", tag="vstg")
                            for sb in range(4):
                                psv = qkps.tile([P, P], F32,
                                                name=f"vps{hh}_{sb}", tag="qk")
                                for cc in range(8):
                                    nc.tensor.matmul(
                                        psv[:], hT[:, cc, sb * P:(sb + 1) * P],
                                        wvt[cc][:, hs],
                                        start=(cc == 0), stop=False)
                                nc.tensor.matmul(psv[:], ones1[:], bvr[:, hs],
                                                 start=False, stop=True)
                                nc.scalar.copy(vstg[:, sb, :], psv[:])
                            nc.sync.dma_start(
                                cb[KN:CH].rearrange(
                                    "(sb p e) -> p sb e", p=P, e=P),
                                vstg[:])
                            # Q chunk -> zero/tiny padded QTA / QTB halves
                            psq = qkps.tile([P, SL], F32, name=f"qps{hh}",
                                            tag="qk")
                            for cc in range(8):
                                nc.tensor.matmul(
                                    psq[:], wqt[cc][:, hs], hT[:, cc, :],
                                    start=(cc == 0), stop=(cc == 7))
                            nc.scalar.activation(
                                QTA[0:64, hh, :], psq[0:64, :], AF.Identity,
                                bias=bq8[0:64, hh:hh + 1], scale=1.0 / 8.0)
                            nc.scalar.activation(
                                QTA[64:128, hh, :], psq[64:128, :],
                                AF.Identity, bias=0.0, scale=TINY / 8.0)
                            nc.scalar.activation(
                                QTB[0:64, hh, :], psq[0:64, :], AF.Identity,
                                bias=0.0, scale=TINY / 8.0)
                            nc.scalar.activation(
                                QTB[64:128, hh, :], psq[64:128, :],
                                AF.Identity, bias=bq8[64:128, hh:hh + 1],
                                scale=1.0 / 8.0)
                            gi = CHUNK_OF[hh]
                            if CHUNK_OFF[hh] == len(CHUNKS[gi]) - 1:
                                nc.gpsimd.collective_compute(
                                    "AllGather", OP.bypass,
                                    replica_groups=groups,
                                    ins=[KVCs[gi].opt()],
                                    outs=[GKVs[gi].opt()])

                # W1/Wo prefetch (DMAs overlap attention)
                wot = []
                for cc in range(8):
                    w = w1pool.tile([P, D], BF16, name=f"wo{cc}", tag="w1")
                    nc.sync.dma_start(w[:], wo_d[cc * P:(cc + 1) * P, :])
                    wot.append(w)
                w1t = [[None] * 8 for _ in range(4)]
                for qq in range(2):
                    for cc in range(8):
                        w = w1pool.tile([P, F // 4], BF16,
                                        name=f"w1_{qq}_{cc}", tag="w1")
                        nc.sync.dma_start(
                            w[:], w1_d[cc * P:(cc + 1) * P,
                                       qq * 1024:(qq + 1) * 1024])
                        w1t[qq][cc] = w

                # ---------------- phase 4: attention ----------------
                ctxT = ctx_pool.tile([P, 8, SL], BF16)
                with (
                    tc.tile_pool(name="kst", bufs=8) as kst,
                    tc.tile_pool(name="vst", bufs=1) as vst,
                    tc.tile_pool(name="esb", bufs=6) as esb,
                    tc.tile_pool(name="bcs_pool", bufs=2) as bcs_pool,
                    tc.tile_pool(name="rs_pool", bufs=1) as rs_pool,
                    tc.tile_pool(name="spsum", bufs=2, space="PSUM") as spsum,
                    tc.tile_pool(name="cpsum", bufs=2, space="PSUM") as cpsum,
                ):
                    # 3 rotating padded V buffers with pre-set ones columns
                    vt_bufs = [vst.tile([P, 4, 2 * P], BF16, name=f"vtb{i}")
                               for i in range(4)]
                    for vb in vt_bufs:
                        nc.vector.memset(vb[:], 0.0)
                        nc.vector.memset(vb[:, :, 64], 1.0)
                        nc.vector.memset(vb[:, :, 192], 1.0)

                    # software-pipelined emission: scores(g+1) is emitted
                    # BEFORE ctx(g) so the in-order PE queue never blocks on
                    # the exp that ctx consumes.
                    cps_all = {}

                    def get_cps(hh, i):
                        key = (hh, i)
                        if key not in cps_all:
                            cps_all[key] = cpsum.tile(
                                [P, SL], F32, name=f"ctx{hh}_{i}",
                                tag=f"ctx{i}")
                        return cps_all[key]

                    kts = {}
                    vt4s = {}

                    def load_kv(hh, c):
                        kt = kst.tile([P, SL], BF16, name=f"kt{hh}_{c}",
                                      tag="kt")
                        gi = CHUNK_OF[hh]
                        csz = len(CHUNKS[gi]) * CH
                        base = c * csz + CHUNK_OFF[hh] * CH
                        ksec = GKVs[gi][base: base + KN].rearrange(
                            "(d q) -> d q", q=SL)
                        nc.sync.dma_start(kt[:], ksec)
                        kts[(hh, c)] = kt
                        vb = vt_bufs[(hh * NCORES + c) % 4]
                        vsec = GKVs[gi][base + KN: base + CH].rearrange(
                            "(s e) -> s e", e=P)
                        nc.sync.dma_start(
                            vb[:, :, 0:64],
                            vsec[:, 0:64].rearrange("(kbl p) e -> p kbl e",
                                                    p=P))
                        nc.sync.dma_start(
                            vb[:, :, 128:192],
                            vsec[:, 64:128].rearrange("(kbl p) e -> p kbl e",
                                                      p=P))
                        vt4s[(hh, c)] = vb

                    steps = [(hh, c, h01, g)
                             for hh in range(8)
                             for c in range(NCORES)
                             for h01 in range(2)
                             for g in range(2)]

                    def emit_scores_exp(step, si):
                        hh, c, h01, g = step
                        if (hh, c) not in kts:
                            load_kv(hh, c)
                        kt = kts[(hh, c)]
                        rhs_q = (QTA if h01 == 0 else QTB)[:, hh, :]
                        sps = spsum.tile([P, 1024], F32,
                                         name=f"sp{hh}_{c}_{h01}_{g}",
                                         tag="sp")
                        for kk in range(2):
                            kbl = g * 2 + kk
                            nc.tensor.matmul(
                                sps[:, kk * 512:(kk + 1) * 512],
                                kt[:, kbl * P:(kbl + 1) * P],
                                rhs_q, start=True, stop=True)
                        et = esb.tile([P, 1024], BF16,
                                      name=f"e{hh}_{c}_{h01}_{g}", tag="et")
                        if (SPILL_MOD and si % SPILL_MOD == SPILL_MOD - 1
                                and si - last_norm_si[0] > 4):
                            # DVE exp: 2^x via exponent-bit construction,
                            # fused scale+bias+f32->i16 convert in one op
                            nc.vector.tensor_scalar(et[:].bitcast(I16),
                                                    sps[:], LOG2E_128,
                                                    EXPBIAS, OP.mult, OP.add)
                        else:
                            nc.scalar.activation(et[:], sps[:], AF.Exp)
                        return (step, et)

                    def emit_ctx(item):
                        (hh, c, h01, g), et = item
                        vb = vt4s[(hh, c)]
                        for kk in range(2):
                            kbl = g * 2 + kk
                            nc.tensor.matmul(
                                get_cps(hh, h01)[:],
                                vb[:, kbl, h01 * P:(h01 + 1) * P],
                                et[:, kk * 512:(kk + 1) * 512],
                                start=(c == 0 and kbl == 0),
                                stop=(c == 7 and kbl == 3))

                    def emit_normalize(hh):
                        cps = [cps_all[(hh, 0)], cps_all[(hh, 1)]]
                        nc.vector.tensor_copy(rc65_f[0:1, :],
                                              cps[0][64:65, :])
                        nc.vector.tensor_copy(rc65_f[64:65, :],
                                              cps[1][64:65, :])
                        rcf = rs_pool.tile([65, SL], F32, name=f"rcf{hh}",
                                           tag="rcf")
                        nc.vector.reciprocal(rcf[:], rc65_f[:])
                        rc65 = rs_pool.tile([65, SL], BF16, name=f"rc{hh}",
                                            tag="rc")
                        nc.vector.tensor_copy(rc65[:], rcf[:])
                        bcw = spsum.tile([P, 1024], F32, name=f"bc{hh}",
                                         tag="sp")
                        bc = bcw[:, 0:SL]
                        nc.tensor.matmul(bc, e65[:], rc65[:], start=True,
                                         stop=True)
                        bcs = bcs_pool.tile([P, SL], F32, name=f"bcs{hh}",
                                            tag="bcs")
                        nc.vector.tensor_copy(bcs[:], bc)
                        nc.vector.tensor_tensor(ctxT[0:64, hh, :],
                                                cps[0][0:64, :],
                                                bcs[0:64, :], OP.mult)
                        nc.vector.tensor_tensor(ctxT[64:128, hh, :],
                                                cps[1][0:64, :],
                                                bcs[64:128, :], OP.mult)

                    norm_q = []
                    pend = []
                    last_norm_si = [-100]
                    for si, step in enumerate(steps):
                        pend.append(emit_scores_exp(step, si))
                        if len(pend) > 3:
                            it = pend.pop(0)
                            emit_ctx(it)
                            phh, pc, ph01, pg = it[0]
                            if pc == 7 and ph01 == 1 and pg == 1:
                                norm_q.append((phh, si + 5))
                        if norm_q and si >= norm_q[0][1]:
                            emit_normalize(norm_q.pop(0)[0])
                            last_norm_si[0] = si
                    for it in pend:
                        emit_ctx(it)
                        phh, pc, ph01, pg = it[0]
                        if pc == 7 and ph01 == 1 and pg == 1:
                            norm_q.append((phh, 0))
                    for hh, _ in norm_q:
                        emit_normalize(hh)

            # ---------------- phase 5: out-proj + residual ----------------
            x2 = x2_pool.tile([P, 4, D], F32)
            with (
                tc.tile_pool(name="ops", bufs=2, space="PSUM") as opps,
                tc.tile_pool(name="hps5", bufs=1, space="PSUM") as hps5,
            ):
                hcast5 = hb_pool.tile([P, SL], BF16, name="hcast5", tag="hb")
                nc.vector.tensor_copy(hcast5[:], ctxT[:, 7, :])
                heat_burst(hps5, 10, hcast5[:], "oproj")
                for sb in range(4):
                    for eb in range(2):
                        ps = opps.tile([P, 512], F32, name=f"op{sb}_{eb}",
                                       tag="op")
                        for cc in range(8):
                            nc.tensor.matmul(
                                ps[:], ctxT[:, cc, sb * P:(sb + 1) * P],
                                wot[cc][:, eb * 512:(eb + 1) * 512],
                                start=(cc == 0), stop=False)
                        nc.tensor.matmul(ps[:], ones65[32:33, :],
                                         bor[:, eb * 512:(eb + 1) * 512],
                                         start=False, stop=True)
                        nc.vector.tensor_tensor(
                            x2[:, sb, eb * 512:(eb + 1) * 512], ps[:],
                            x_big[:, sb, eb * 512:(eb + 1) * 512], OP.add)

        # ---------------- phase 6: LN2 + transpose ----------------
        with tc.tile_pool(name="h2T_pool", bufs=1) as h2T_pool:
            h2T = h2T_pool.tile([P, 8, SL], BF16)
            with tc.tile_pool(name="tp2", bufs=2, space="PSUM") as tpp2:
                layer_norm_to_T(x2, ln2_a, ln2_b, h2T, tpp2, burst_pool=tpp2)

            # ------------- phases 7/8: FFN in two halves -------------
            with (
                tc.tile_pool(name="atpool", bufs=4) as atpool,
                tc.tile_pool(name="w2pool", bufs=18) as w2pool,
                tc.tile_pool(name="o2ppool", bufs=1) as o2ppool,
                tc.tile_pool(name="outpool", bufs=3) as outpool,
                tc.tile_pool(name="f1ps", bufs=2, space="PSUM") as f1ps,
                tc.tile_pool(name="f2ps", bufs=4, space="PSUM") as f2ps,
            ):
                o2p = o2ppool.tile([P, 4, D], F32)
                with tc.tile_pool(name="hps7", bufs=1, space="PSUM") as hps7:
                    hcast7 = hb_pool.tile([P, SL], BF16, name="hcast7",
                                          tag="hb")
                    nc.vector.tensor_copy(hcast7[:], h2T[:, 0, :])
                    heat_burst(hps7, 10, hcast7[:], "ffn")
                for qq in range(2, 4):
                    for cc in range(8):
                        w = w1pool.tile([P, F // 4], BF16,
                                        name=f"w1_{qq}_{cc}", tag="w1")
                        nc.sync.dma_start(
                            w[:], w1_d[cc * P:(cc + 1) * P,
                                       qq * 1024:(qq + 1) * 1024])
                        w1t[qq][cc] = w
                for half in range(2):
                    at_h = []
                    for qq in range(half * 2, half * 2 + 2):
                        ATq = atpool.tile([P, 8, SL], BF16,
                                          name=f"at{qq}", tag="at")
                        for fc in range(8):
                            fg = qq * 8 + fc
                            ps = f1ps.tile([P, SL], F32, name=f"f1_{fg}",
                                           tag="f1")
                            for cc in range(8):
                                nc.tensor.matmul(
                                    ps[:],
                                    w1t[qq][cc][:, fc * P:(fc + 1) * P],
                                    h2T[:, cc, :], start=(cc == 0),
                                    stop=(cc == 7))
                            nc.vector.tensor_scalar(ATq[:, fc, :], ps[:],
                                                    b1_t[:, fg:fg + 1],
                                                    0.0, OP.add, OP.max)
                        at_h.append(ATq)
                    w2ts = []
                    for fcl in range(16):
                        fg = half * 16 + fcl
                        w2t = w2pool.tile([P, D], BF16, name=f"w2_{fg}",
                                          tag="w2")
                        nc.sync.dma_start(w2t[:],
                                          w2_d[fg * P:(fg + 1) * P, :])
                        w2ts.append(w2t)
                    for eb in range(2):
                        sl = slice(eb * 512, (eb + 1) * 512)
                        pss = [f2ps.tile([P, 512], F32,
                                         name=f"f2_{half}_{eb}_{sb}",
                                         tag="f2") for sb in range(4)]
                        for fcl in range(16):
                            qq, fc = divmod(fcl, 8)
                            for sb in range(4):
                                nc.tensor.matmul(
                                    pss[sb][:],
                                    at_h[qq][:, fc, sb * P:(sb + 1) * P],
                                    w2ts[fcl][:, sl],
                                    start=(fcl == 0),
                                    stop=(half == 0 and fcl == 15))
                        for sb in range(4):
                            ps = pss[sb]
                            if half == 0:
                                nc.vector.tensor_tensor(
                                    o2p[:, sb, sl], ps[:], x2[:, sb, sl],
                                    OP.add)
                            else:
                                nc.tensor.matmul(ps[:], ones65[64:65, :],
                                                 b2r[:, sl],
                                                 start=False, stop=True)
                                ot = outpool.tile([P, 512], F32,
                                                  name=f"ot{sb}_{eb}",
                                                  tag="ot")
                                nc.vector.tensor_tensor(ot[:], ps[:],
                                                        o2p[:, sb, sl],
                                                        OP.add)
                                nc.sync.dma_start(
                                    y_d[sb * P:(sb + 1) * P, sl], ot[:])

    nc.compile()
    return nc


def kernel(**inputs):
    import ml_dtypes
    BF = ml_dtypes.bfloat16
    inp = {k: np.asarray(v, dtype=np.float32) for k, v in inputs.items()}
    x = inp["x"]
    B = x.shape[0]
    key = (float(inp["ln1_a"][0]), float(inp["ln1_b"][0]),
           float(inp["ln2_a"][0]), float(inp["ln2_b"][0]))
    if key not in _CACHE:
        _CACHE[key] = _build(*key)
    nc = _CACHE[key]

    xf = x.reshape(S, D)
    shared = {
        "Wq": inp["Wq"].astype(BF), "Wk": inp["Wk"].astype(BF),
        "Wv": inp["Wv"].astype(BF), "Wo": inp["Wo"].astype(BF),
        "W1": inp["W1"].astype(BF), "W2": inp["W2"].astype(BF),
        "bq": inp["bq"], "bk": inp["bk"], "b1": inp["b1"],
        "bx3": np.stack([inp["bv"], inp["bo"], inp["b2"]]).astype(BF),
    }
    in_maps = []
    for c in range(NCORES):
        m = dict(shared)
        m["x_loc"] = np.ascontiguousarray(xf[c * SL:(c + 1) * SL, :])
        in_maps.append(m)
    res = run_bass_kernel_spmd(nc, in_maps, list(range(NCORES)))
    out = np.concatenate([res.results[c]["y_loc"] for c in range(NCORES)],
                         axis=0)
    return out.reshape(B, S, D)
